# revision 3
# baseline (speedup 1.0000x reference)
"""Trainium2 Bass kernel v2: 3x3 valid conv, x(16,2048,2048) f32 -> y(16,2046,2046) f32.

Same core design as v1 (band-Toeplitz over 8-row tiles, fp8-e3m4 x /
fp16 w+y, dx-outer matmuls, parity-alternated out partitions), plus:
  - warm-up matmuls on an uninitialized SBUF tile (no DMA dependency)
    ramp the PE out of its low p-state during the input-DMA head;
  - tile0's x DMA is split so the first matmul only waits for the
    first 516 columns;
  - 42x6+4 tiles = exactly 256 output rows (v1 computed 258);
  - the last tiles ship each PSUM chunk as its own DMA, alternating
    queues, to shorten the post-compute tail.
"""

import sys

sys.path.insert(0, "/opt/trn_rl_repo")

import numpy as np
import ml_dtypes

NCORES = 8
CIN = 16
COUT = 16
H = 2048
W = 2048
HOUT = 2046
WOUT = 2046
ROWS_PER_CORE = 256
TILE_IN = 8
TILE_OUT = 6

FULL_TILES = [6] * 42 + [4]  # 256 output rows
FULL_CHUNKS = [(0, 512), (512, 512), (1024, 512), (1536, 510)]

E3 = ml_dtypes.float8_e3m4


def build_conv_bass(
    tiles, w_in, chunks, dt_x, dt_w, dt_out, num_devices=NCORES,
    xbufs=8, obufs=8, warm_mms=8, split_x0=True, chunk_odma_tail=2,
):
    """tiles: list of output rows per tile (6 = full band, 4 = short last)."""
    from concourse import bacc, tile, mybir

    n_tiles = len(tiles)
    h_in = sum(tiles) + 2
    h_out = sum(tiles)
    w_out = max(c0 + cw for c0, cw in chunks)
    chunk_max = max(cw for _, cw in chunks)

    nc = bacc.Bacc(
        "TRN2",
        target_bir_lowering=False,
        debug=False,
        enable_asserts=False,
        num_devices=num_devices,
    )
    xs = nc.dram_tensor("xs", [h_in, CIN, w_in], dt_x, kind="ExternalInput")
    wt = nc.dram_tensor("wt", [128, 3, 2, 128], dt_w, kind="ExternalInput")
    y = nc.dram_tensor("y", [h_out, COUT, w_out], dt_out, kind="ExternalOutput")
    xs_ap = xs.ap()
    wt_ap = wt.ap()
    y_ap = y.ap()

    with tile.TileContext(nc) as tc:
        with (
            tc.tile_pool(name="wpool", bufs=1) as wpool,
            tc.tile_pool(name="xpool", bufs=xbufs) as xpool,
            tc.tile_pool(name="opool", bufs=obufs) as opool,
            tc.tile_pool(name="psum", bufs=2, space="PSUM") as ppool,
        ):
            # PE warm-up: matmuls on a never-written SBUF tile have no DMA
            # dependency, so they start as soon as the PE sequencer is up and
            # ramp the clock out of the low p-state while x0 is in flight.
            if warm_mms:
                wu = wpool.tile([128, 512], dt_w, name="warm")
                nc.vector.memset(wu[:], 1.0)
                psw = ppool.tile([128, chunk_max], mybir.dt.float32, name="ps0")
                for r in range(warm_mms):
                    nc.tensor.matmul(
                        psw[:, :512],
                        wu[:, :128],
                        wu[:, :512],
                        start=(r == 0),
                        stop=(r == warm_mms - 1),
                    )

            x0_tile = xpool.tile([128, w_in], dt_x, name="x_tile")
            if split_x0:
                c_split = chunks[0][1] + 4  # 516
                nc.sync.dma_start(
                    x0_tile[:, :c_split],
                    xs_ap[0:TILE_IN, :, :c_split].rearrange("g c w -> (g c) w"),
                )
                nc.sync.dma_start(
                    x0_tile[:, c_split:],
                    xs_ap[0:TILE_IN, :, c_split:].rearrange("g c w -> (g c) w"),
                )
            else:
                nc.sync.dma_start(
                    x0_tile[:], xs_ap[0:TILE_IN].rearrange("g c w -> (g c) w")
                )
            w_tile = wpool.tile([128, 3, 2, 128], dt_w)
            nc.sync.dma_start(w_tile[:], wt_ap[:])

            r0 = 0  # output row of tile start
            for t, t_rows in enumerate(tiles):
                par = t % 2
                p0 = 32 * par
                g_in = t_rows + 2  # input row-groups this tile needs
                if t == 0:
                    x_tile = x0_tile
                else:
                    x_tile = xpool.tile([128, w_in], dt_x, name="x_tile")
                    nc.sync.dma_start(
                        x_tile[: 16 * g_in],
                        xs_ap[r0 : r0 + g_in].rearrange("g c w -> (g c) w"),
                    )
                o_tile = opool.tile([128, w_out], dt_out)
                y_rows = y_ap[r0 : r0 + t_rows].rearrange("g c w -> (g c) w")
                pss = [
                    ppool.tile([128, chunk_max], mybir.dt.float32, name=f"ps{ci}")
                    for ci in range(len(chunks))
                ]
                if t >= n_tiles - chunk_odma_tail:
                    # tail tiles: chunk-major matmul order so each chunk
                    # finishes 3 matmuls in, and its drain + DMA pipeline
                    # under the tile's remaining matmuls (shorter tail);
                    # alternating engines/queues so the last ships overlap.
                    for ci, (c0, cw) in enumerate(chunks):
                        for dx in range(3):
                            nc.tensor.matmul(
                                pss[ci][:, :cw],
                                w_tile[:, dx, par, :],
                                x_tile[:, c0 + dx : c0 + dx + cw],
                                start=(dx == 0),
                                stop=(dx == 2),
                            )
                        if ci % 2 == 0:
                            nc.scalar.copy(o_tile[:, c0 : c0 + cw], pss[ci][:, :cw])
                        else:
                            nc.vector.tensor_copy(
                                o_tile[:, c0 : c0 + cw], pss[ci][:, :cw]
                            )
                        q = nc.gpsimd if ci % 2 == 0 else nc.sync
                        q.dma_start(
                            y_rows[:, c0 : c0 + cw],
                            o_tile[p0 : p0 + 16 * t_rows, c0 : c0 + cw],
                        )
                else:
                    for dx in range(3):
                        for ci, (c0, cw) in enumerate(chunks):
                            nc.tensor.matmul(
                                pss[ci][:, :cw],
                                w_tile[:, dx, par, :],
                                x_tile[:, c0 + dx : c0 + dx + cw],
                                start=(dx == 0),
                                stop=(dx == 2),
                            )
                    for ci, (c0, cw) in enumerate(chunks):
                        if ci % 2 == 0:
                            nc.scalar.copy(o_tile[:, c0 : c0 + cw], pss[ci][:, :cw])
                        else:
                            nc.vector.tensor_copy(
                                o_tile[:, c0 : c0 + cw], pss[ci][:, :cw]
                            )
                    nc.gpsimd.dma_start(y_rows, o_tile[p0 : p0 + 16 * t_rows, :])
                r0 += t_rows

    nc.compile()
    return nc


def pack_weights(kernels, np_dt):
    """kernels (16,16,3,3) -> band-Toeplitz lhsT [128, 3, 2, 128]."""
    wnp = np.zeros((128, 3, 2, 128), np_dt)
    k = np.asarray(kernels, np.float32)
    for g in range(TILE_IN):
        for gp in range(max(0, g - 2), min(g + 1, TILE_OUT)):
            dy = g - gp
            blk = k[:, :, dy, :].transpose(1, 2, 0).astype(np_dt)  # [ci, dx, co]
            for par in range(2):
                m0 = 32 * par + gp * 16
                wnp[g * 16 : (g + 1) * 16, :, par, m0 : m0 + 16] = blk
    return wnp


def make_in_maps(x, kernels, np_x, np_w):
    h_in = sum(FULL_TILES) + 2  # 258
    wnp = pack_weights(kernels, np_w)
    x = np.asarray(x)
    in_maps = []
    for c in range(NCORES):
        r0 = ROWS_PER_CORE * c
        r1 = min(r0 + h_in, H)
        rows = r1 - r0
        xs = np.zeros((h_in, CIN, W), np_x)
        xs[:rows] = x[:, r0:r1, :].transpose(1, 0, 2).astype(np_x, copy=False)
        in_maps.append({"xs": xs, "wt": wnp})
    return in_maps


def assemble_output(results):
    out = np.empty((COUT, HOUT, WOUT), np.float32)
    for c in range(NCORES):
        yc = results[c]["y"]  # [256, 16, 2046]
        rows = min(ROWS_PER_CORE, HOUT - ROWS_PER_CORE * c)
        out[:, ROWS_PER_CORE * c : ROWS_PER_CORE * c + rows, :] = yc[:rows].transpose(
            1, 0, 2
        )
    return out


_CACHE = {}


def dtype_config(dtype):
    from concourse import mybir

    if dtype == "e3mix":
        return (E3, np.float16, mybir.dt.float8e3, mybir.dt.float16,
                mybir.dt.float16)
    if dtype == "float16":
        return (np.float16, np.float16, mybir.dt.float16, mybir.dt.float16,
                mybir.dt.float16)
    raise ValueError(dtype)


def run_conv(x, kernels, dtype="e3mix", trace=False):
    from concourse import bass_utils

    np_x, np_w, dt_x, dt_w, dt_out = dtype_config(dtype)

    if dtype not in _CACHE:
        _CACHE[dtype] = build_conv_bass(
            FULL_TILES, W, FULL_CHUNKS, dt_x, dt_w, dt_out
        )
    nc = _CACHE[dtype]

    in_maps = make_in_maps(x, kernels, np_x, np_w)
    res = bass_utils.run_bass_kernel_spmd(
        nc, in_maps, core_ids=list(range(NCORES)), trace=trace
    )
    return assemble_output(res.results), res


def kernel(x, kernels):
    out, _ = run_conv(x, kernels, dtype="e3mix", trace=False)
    return out
